# revision 6
# baseline (speedup 1.0000x reference)
"""Trainium2 Bass kernel for nn_DecoderRNN: 1024-step LSTM decoder with greedy
argmax feedback, batch=1, H=2048, V=7.

Strategy (8 NeuronCores, tensor-parallel over the 4H gate dim):
  - Core k owns hidden slice [256k, 256k+256) of every gate. Its [1024 x 2176]
    augmented weight block (W_hh | W_ih | fused bias) stays SBUF-resident in
    bf16 for the whole run; the recurrent GEMV streams it through the PE as the
    moving operand (h stationary), accumulating fp32 in PSUM.
  - The one-hot token feedback is folded into the GEMV as an extra contraction
    chunk: moving vector = [h (2048) | onehot (7) | 1 (bias) | 0...].
  - Each step all-gathers the 8 h-slices (bf16) through a DRAM bounce; the
    gathered vector is transposed back to SBUF column layout with two PE
    transposes. Logits, argmax -> one-hot, and the log-softmax epilogue all run
    on-device; argmax feedback matches the fp32 reference exactly (top-2 logit
    gap ~7.5e-3 vs bf16 logit error ~1.3e-3).
"""
import sys
sys.path.insert(0, '/opt/trn_rl_repo')
import numpy as np
import ml_dtypes

H = 2048
V = 7
T = 1024
NCORES = 8
S = H // NCORES          # 256
GPC = 4 * S              # 1024 gate rows per core
KC = 17                  # contraction chunks: 16 of h + 1 aug

_cache = {}


def _pack_inputs(inp, core):
    W_ih = np.asarray(inp["W_ih"], np.float32)
    W_hh = np.asarray(inp["W_hh"], np.float32)
    b = np.asarray(inp["b_ih"], np.float32) + np.asarray(inp["b_hh"], np.float32)
    W_out = np.asarray(inp["W_out"], np.float32)
    b_out = np.asarray(inp["b_out"], np.float32)
    h0 = np.asarray(inp["h0"], np.float32)[0]
    c0 = np.asarray(inp["c0"], np.float32)[0]

    rows = np.concatenate([g * H + np.arange(core * S, (core + 1) * S)
                           for g in range(4)])
    W_aug = np.zeros((GPC, KC * 128), np.float32)
    W_aug[:, :H] = W_hh[rows]
    W_aug[:, H:H + V] = W_ih[rows]
    W_aug[:, H + V] = b[rows]
    Wt = W_aug.T.reshape(KC, 128, GPC).transpose(1, 0, 2).reshape(128, KC * GPC)
    Wt = Wt.astype(ml_dtypes.bfloat16)

    Wo_aug = np.zeros((KC * 128, V), np.float32)
    Wo_aug[:H] = W_out.T
    Wo_aug[H + V] = b_out
    Wo = Wo_aug.reshape(KC, 128, V).transpose(1, 0, 2).reshape(128, KC * V)
    Wo = Wo.astype(ml_dtypes.bfloat16)

    h0c = h0.reshape(16, 128).T.astype(ml_dtypes.bfloat16).copy()
    x0c = np.zeros((128, 1), ml_dtypes.bfloat16)
    x0c[V - 1, 0] = 1.0
    x0c[V, 0] = 1.0
    c0r = c0[core * S:(core + 1) * S].reshape(1, S).astype(np.float32)
    return {"Wt": Wt, "Wo": Wo, "h0c": h0c, "x0c": x0c, "c0r": c0r,
            "id8": np.eye(8, dtype=ml_dtypes.bfloat16)}


def _build_nc():
    from concourse import bacc, tile, mybir
    BF16 = mybir.dt.bfloat16
    F32 = mybir.dt.float32

    nc = bacc.Bacc("TRN2", target_bir_lowering=False, debug=False,
                   enable_asserts=True, num_devices=NCORES)
    wt_d = nc.dram_tensor("Wt", [128, KC * GPC], BF16, kind="ExternalInput")
    wo_d = nc.dram_tensor("Wo", [128, KC * V], BF16, kind="ExternalInput")
    h0c_d = nc.dram_tensor("h0c", [128, 16], BF16, kind="ExternalInput")
    x0c_d = nc.dram_tensor("x0c", [128, 1], BF16, kind="ExternalInput")
    c0r_d = nc.dram_tensor("c0r", [1, S], F32, kind="ExternalInput")
    id8_d = nc.dram_tensor("id8", [8, 8], BF16, kind="ExternalInput")
    out_lp = nc.dram_tensor("out_lp", [T, V], F32, kind="ExternalOutput")
    out_h = nc.dram_tensor("out_h", [1, H], F32, kind="ExternalOutput")
    out_c = nc.dram_tensor("out_c", [1, H], F32, kind="ExternalOutput")

    with tile.TileContext(nc) as tc:
        with tc.tile_pool(name="persist", bufs=1) as pp, \
             tc.tile_pool(name="scratch", bufs=2) as sp, \
             tc.tile_pool(name="psum", bufs=2, space="PSUM") as psp, \
             tc.tile_pool(name="post", bufs=1) as postp, \
             tc.tile_pool(name="dram", bufs=1, space="DRAM") as dp:

            wt = pp.tile([128, KC * GPC], BF16, tag="wt")
            wo = pp.tile([128, KC * V], BF16, tag="wo")
            ha = [pp.tile([128, KC], BF16, tag=f"ha{i}", name=f"ha{i}")
                  for i in range(2)]
            crow = pp.tile([1, S], F32, tag="crow")
            lstore = pp.tile([1, T, V], F32, tag="lstore")
            ident = pp.tile([1, 1], BF16, tag="ident")
            id8 = pp.tile([8, 8], BF16, tag="id8")

            nc.sync.dma_start(wt[:], wt_d[:])
            nc.sync.dma_start(wo[:], wo_d[:])
            nc.sync.dma_start(ha[0][:, 0:16], h0c_d[:])
            nc.sync.dma_start(ha[0][:, 16:17], x0c_d[:])
            nc.sync.dma_start(ha[1][:, 16:17], x0c_d[:])
            nc.sync.dma_start(crow[:], c0r_d[:])
            nc.sync.dma_start(id8[:], id8_d[:])
            nc.vector.memset(ident[:], 1.0)

            agin = [dp.tile([1, S], BF16, tag=f"agin{i}", name=f"agin{i}")
                    for i in range(2)]
            agout = [dp.tile([8, S], BF16, tag=f"agout{i}", name=f"agout{i}")
                     for i in range(2)]
            fin_in = dp.tile([1, 2 * S], F32, tag="fin_in")
            fin_out = dp.tile([8, 2 * S], F32, tag="fin_out", addr_space="Shared")

            Sig = mybir.ActivationFunctionType.Sigmoid
            Tanh = mybir.ActivationFunctionType.Tanh
            for t in range(T):
                par = t % 2
                hcur, hnxt = ha[par], ha[1 - par]
                gp = [psp.tile([1, S], F32, tag=f"g{i}", name=f"g{i}", bufs=1)
                      for i in range(4)]
                # bank-major order f,i,g,o: each gate's PSUM bank completes at a
                # quarter boundary of the PE stream, so its ACT/DVE work overlaps
                # the remaining matmuls (f first: it feeds sf*c on the c-chain)
                for j in (1, 0, 2, 3):
                    for c in range(KC):
                        nc.tensor.matmul(
                            gp[j][0:1, :],
                            lhsT=hcur[:, c:c + 1],
                            rhs=wt[:, GPC * c + S * j: GPC * c + S * (j + 1)],
                            start=(c == 0), stop=(c == KC - 1),
                        )
                ap_i = gp[0][0:1, :]
                ap_f = gp[1][0:1, :]
                ap_g = gp[2][0:1, :]
                ap_o = gp[3][0:1, :]
                sf = sp.tile([1, S], F32, tag="sf")
                si = sp.tile([1, S], F32, tag="si")
                tg = sp.tile([1, S], F32, tag="tg")
                so = sp.tile([1, S], F32, tag="so")
                t1 = sp.tile([1, S], F32, tag="t1")
                t2 = sp.tile([1, S], F32, tag="t2")
                tc2 = sp.tile([1, S], F32, tag="tc2")
                h2b = sp.tile([1, S], BF16, tag="h2b")
                nc.scalar.activation(sf[:], ap_f, Sig)
                nc.vector.tensor_tensor(t2[:], sf[:], crow[:], mybir.AluOpType.mult)
                nc.scalar.activation(si[:], ap_i, Sig)
                nc.scalar.activation(tg[:], ap_g, Tanh)
                nc.vector.tensor_tensor(t1[:], si[:], tg[:], mybir.AluOpType.mult)
                nc.vector.tensor_tensor(crow[:], t1[:], t2[:], mybir.AluOpType.add)
                nc.scalar.activation(tc2[:], crow[:], Tanh)
                nc.scalar.activation(so[:], ap_o, Sig)
                nc.vector.tensor_tensor(h2b[:], so[:], tc2[:], mybir.AluOpType.mult)

                nc.sync.dma_start(agin[par][:], h2b[:])
                nc.gpsimd.collective_compute(
                    "AllGather", mybir.AluOpType.bypass,
                    replica_groups=[list(range(NCORES))],
                    ins=[agin[par][:].opt()], outs=[agout[par][:].opt()],
                )
                g8 = sp.tile([8, S], BF16, tag="g8")
                nc.sync.dma_start(g8[:], agout[par][:])
                tp = psp.tile([128, 16], BF16, tag="tp", bufs=1)
                nc.tensor.transpose(tp[:, 0:8], g8[0:8, 0:128], id8[:])
                nc.tensor.transpose(tp[:, 8:16], g8[0:8, 128:256], id8[:])
                nc.vector.tensor_copy(hnxt[:, 0:16:2], tp[:, 0:8])
                nc.vector.tensor_copy(hnxt[:, 1:16:2], tp[:, 8:16])

                lp = psp.tile([1, V], F32, tag="lp", bufs=1)
                for c in range(KC):
                    nc.tensor.matmul(
                        lp[0:1, :],
                        lhsT=hnxt[:, c:c + 1],
                        rhs=wo[:, V * c: V * (c + 1)],
                        start=(c == 0), stop=(c == KC - 1),
                    )
                nc.vector.tensor_copy(lstore[0:1, t, :], lp[0:1, :])
                mx = sp.tile([1, 1], F32, tag="mx")
                ohrow = sp.tile([1, V], BF16, tag="ohrow")
                ohcol = psp.tile([V, 1], BF16, tag="ohcol", bufs=1)
                nc.vector.reduce_max(mx[:], lp[0:1, :], mybir.AxisListType.X)
                nc.vector.tensor_scalar(ohrow[:], lp[0:1, :], mx[:], None,
                                        mybir.AluOpType.is_equal)
                nc.tensor.transpose(ohcol[:], ohrow[:], ident[:])
                nc.vector.tensor_copy(hnxt[0:V, 16:17], ohcol[:])

                if t == T - 1:
                    h2f = sp.tile([1, S], F32, tag="h2f")
                    nc.vector.tensor_tensor(h2f[:], so[:], tc2[:],
                                            mybir.AluOpType.mult)
                    nc.sync.dma_start(fin_in[0:1, 0:S], h2f[:])
                    nc.sync.dma_start(fin_in[0:1, S:2 * S], crow[:])

            nc.gpsimd.collective_compute(
                "AllGather", mybir.AluOpType.bypass,
                replica_groups=[list(range(NCORES))],
                ins=[fin_in[:].opt()], outs=[fin_out[:].opt()],
            )
            nc.sync.dma_start(out_h[:].rearrange("a (r s) -> (a r) s", s=S),
                              fin_out[:, 0:S])
            nc.sync.dma_start(out_c[:].rearrange("a (r s) -> (a r) s", s=S),
                              fin_out[:, S:2 * S])

            mxs = postp.tile([1, T, 1], F32, tag="mxs")
            sub = postp.tile([1, T, V], F32, tag="sub")
            ex = postp.tile([1, T, V], F32, tag="ex")
            sm = postp.tile([1, T, 1], F32, tag="sm")
            lsm = postp.tile([1, T, 1], F32, tag="lsm")
            nc.vector.reduce_max(mxs[0:1, :, 0], lstore[:], mybir.AxisListType.X)
            nc.vector.tensor_tensor(sub[:], lstore[:],
                                    mxs[:].broadcast_to([1, T, V]),
                                    mybir.AluOpType.subtract)
            nc.scalar.activation(ex[:], sub[:], mybir.ActivationFunctionType.Exp)
            nc.vector.reduce_sum(sm[0:1, :, 0], ex[:], mybir.AxisListType.X)
            nc.scalar.activation(lsm[:], sm[:], mybir.ActivationFunctionType.Ln)
            nc.vector.tensor_tensor(sub[:], sub[:],
                                    lsm[:].broadcast_to([1, T, V]),
                                    mybir.AluOpType.subtract)
            nc.sync.dma_start(out_lp[:].rearrange("t v -> (t v)").unsqueeze(0),
                              sub[0:1, :, :])

    nc.compile()
    return nc


def _get_nc():
    if "nc" not in _cache:
        _cache["nc"] = _build_nc()
    return _cache["nc"]


def run_on_device(inputs, trace=False, tmpdir=None):
    from concourse import bass_utils
    nc = _get_nc()
    in_maps = [_pack_inputs(inputs, core) for core in range(NCORES)]
    res = bass_utils.run_bass_kernel_spmd(
        nc, in_maps, core_ids=list(range(NCORES)), trace=trace, tmpdir=tmpdir)
    r0 = res.results[0]
    out = (np.asarray(r0["out_lp"], np.float32),
           np.asarray(r0["out_h"], np.float32),
           np.asarray(r0["out_c"], np.float32))
    return out, res


def kernel(**inputs):
    out, _ = run_on_device(inputs, trace=False)
    return out


# revision 7
# speedup vs baseline: 1.0130x; 1.0130x over previous
"""Trainium2 Bass kernel for nn_DecoderRNN: 1024-step LSTM decoder with greedy
argmax feedback, batch=1, H=2048, V=7.

Strategy (8 NeuronCores, tensor-parallel over the 4H gate dim):
  - Core k owns hidden slice [256k, 256k+256) of every gate. Its [1024 x 2176]
    augmented weight block (W_hh | W_ih | fused bias) stays SBUF-resident in
    bf16 for the whole run; the recurrent GEMV streams it through the PE as the
    moving operand (h stationary), accumulating fp32 in PSUM.
  - The one-hot token feedback is folded into the GEMV as an extra contraction
    chunk: moving vector = [h (2048) | onehot (7) | 1 (bias) | 0...].
  - Each step all-gathers the 8 h-slices (bf16) through a DRAM bounce; the
    gathered vector is transposed back to SBUF column layout with two PE
    transposes. Logits, argmax -> one-hot, and the log-softmax epilogue all run
    on-device; argmax feedback matches the fp32 reference exactly (top-2 logit
    gap ~7.5e-3 vs bf16 logit error ~1.3e-3).
"""
import sys
sys.path.insert(0, '/opt/trn_rl_repo')
import numpy as np
import ml_dtypes

H = 2048
V = 7
T = 1024
NCORES = 8
S = H // NCORES          # 256
GPC = 4 * S              # 1024 gate rows per core
KC = 17                  # contraction chunks: 16 of h + 1 aug

_cache = {}


def _pack_inputs(inp, core):
    W_ih = np.asarray(inp["W_ih"], np.float32)
    W_hh = np.asarray(inp["W_hh"], np.float32)
    b = np.asarray(inp["b_ih"], np.float32) + np.asarray(inp["b_hh"], np.float32)
    W_out = np.asarray(inp["W_out"], np.float32)
    b_out = np.asarray(inp["b_out"], np.float32)
    h0 = np.asarray(inp["h0"], np.float32)[0]
    c0 = np.asarray(inp["c0"], np.float32)[0]

    rows = np.concatenate([g * H + np.arange(core * S, (core + 1) * S)
                           for g in range(4)])
    W_aug = np.zeros((GPC, KC * 128), np.float32)
    W_aug[:, :H] = W_hh[rows]
    W_aug[:, H:H + V] = W_ih[rows]
    W_aug[:, H + V] = b[rows]
    Wt = W_aug.T.reshape(KC, 128, GPC).transpose(1, 0, 2).reshape(128, KC * GPC)
    Wt = Wt.astype(ml_dtypes.bfloat16)

    Wo_aug = np.zeros((KC * 128, V), np.float32)
    Wo_aug[:H] = W_out.T
    Wo_aug[H + V] = b_out
    Wo = Wo_aug.reshape(KC, 128, V).transpose(1, 0, 2).reshape(128, KC * V)
    Wo = Wo.astype(ml_dtypes.bfloat16)

    h0c = h0.reshape(16, 128).T.astype(ml_dtypes.bfloat16).copy()
    x0c = np.zeros((128, 1), ml_dtypes.bfloat16)
    x0c[V - 1, 0] = 1.0
    x0c[V, 0] = 1.0
    c0r = c0[core * S:(core + 1) * S].reshape(1, S).astype(np.float32)
    return {"Wt": Wt, "Wo": Wo, "h0c": h0c, "x0c": x0c, "c0r": c0r,
            "id8": np.eye(8, dtype=ml_dtypes.bfloat16)}


def _build_nc():
    from concourse import bacc, tile, mybir
    BF16 = mybir.dt.bfloat16
    F32 = mybir.dt.float32

    nc = bacc.Bacc("TRN2", target_bir_lowering=False, debug=False,
                   enable_asserts=True, num_devices=NCORES)
    wt_d = nc.dram_tensor("Wt", [128, KC * GPC], BF16, kind="ExternalInput")
    wo_d = nc.dram_tensor("Wo", [128, KC * V], BF16, kind="ExternalInput")
    h0c_d = nc.dram_tensor("h0c", [128, 16], BF16, kind="ExternalInput")
    x0c_d = nc.dram_tensor("x0c", [128, 1], BF16, kind="ExternalInput")
    c0r_d = nc.dram_tensor("c0r", [1, S], F32, kind="ExternalInput")
    id8_d = nc.dram_tensor("id8", [8, 8], BF16, kind="ExternalInput")
    out_lp = nc.dram_tensor("out_lp", [T, V], F32, kind="ExternalOutput")
    out_h = nc.dram_tensor("out_h", [1, H], F32, kind="ExternalOutput")
    out_c = nc.dram_tensor("out_c", [1, H], F32, kind="ExternalOutput")

    with tile.TileContext(nc) as tc:
        with tc.tile_pool(name="persist", bufs=1) as pp, \
             tc.tile_pool(name="scratch", bufs=2) as sp, \
             tc.tile_pool(name="psum", bufs=2, space="PSUM") as psp, \
             tc.tile_pool(name="post", bufs=1) as postp, \
             tc.tile_pool(name="dram", bufs=1, space="DRAM") as dp:

            wt = pp.tile([128, KC * GPC], BF16, tag="wt")
            wo = pp.tile([128, KC * V], BF16, tag="wo")
            ha = [pp.tile([128, KC], BF16, tag=f"ha{i}", name=f"ha{i}")
                  for i in range(2)]
            crow = pp.tile([1, S], F32, tag="crow")
            lstore = pp.tile([1, T, V], F32, tag="lstore")
            ident = pp.tile([1, 1], BF16, tag="ident")
            id8 = pp.tile([8, 8], BF16, tag="id8")

            nc.sync.dma_start(wt[:], wt_d[:])
            nc.sync.dma_start(wo[:], wo_d[:])
            nc.sync.dma_start(ha[0][:, 0:16], h0c_d[:])
            nc.sync.dma_start(ha[0][:, 16:17], x0c_d[:])
            nc.sync.dma_start(ha[1][:, 16:17], x0c_d[:])
            nc.sync.dma_start(crow[:], c0r_d[:])
            nc.sync.dma_start(id8[:], id8_d[:])
            nc.vector.memset(ident[:], 1.0)

            agin = [dp.tile([1, S], BF16, tag=f"agin{i}", name=f"agin{i}")
                    for i in range(2)]
            agout = [dp.tile([8, S], BF16, tag=f"agout{i}", name=f"agout{i}")
                     for i in range(2)]
            fin_in = dp.tile([1, 2 * S], F32, tag="fin_in")
            fin_out = dp.tile([8, 2 * S], F32, tag="fin_out", addr_space="Shared")

            Sig = mybir.ActivationFunctionType.Sigmoid
            Tanh = mybir.ActivationFunctionType.Tanh
            for t in range(T):
                par = t % 2
                hcur, hnxt = ha[par], ha[1 - par]
                gp = [psp.tile([1, S], F32, tag=f"g{i}", name=f"g{i}", bufs=1)
                      for i in range(4)]
                # bank-major order f,i,g,o: each gate's PSUM bank completes at a
                # quarter boundary of the PE stream, so its ACT/DVE work overlaps
                # the remaining matmuls (f first: it feeds sf*c on the c-chain)
                for j in (1, 0, 2, 3):
                    for c in range(KC):
                        nc.tensor.matmul(
                            gp[j][0:1, :],
                            lhsT=hcur[:, c:c + 1],
                            rhs=wt[:, GPC * c + S * j: GPC * c + S * (j + 1)],
                            start=(c == 0), stop=(c == KC - 1),
                        )
                ap_i = gp[0][0:1, :]
                ap_f = gp[1][0:1, :]
                ap_g = gp[2][0:1, :]
                ap_o = gp[3][0:1, :]
                sf = sp.tile([1, S], F32, tag="sf")
                si = sp.tile([1, S], F32, tag="si")
                tg = sp.tile([1, S], F32, tag="tg")
                so = sp.tile([1, S], F32, tag="so")
                t1 = sp.tile([1, S], F32, tag="t1")
                t2 = sp.tile([1, S], F32, tag="t2")
                tc2 = sp.tile([1, S], F32, tag="tc2")
                h2b = sp.tile([1, S], BF16, tag="h2b")
                nc.scalar.activation(sf[:], ap_f, Sig)
                nc.vector.tensor_tensor(t2[:], sf[:], crow[:], mybir.AluOpType.mult)
                nc.scalar.activation(si[:], ap_i, Sig)
                nc.scalar.activation(tg[:], ap_g, Tanh)
                nc.vector.tensor_tensor(t1[:], si[:], tg[:], mybir.AluOpType.mult)
                nc.vector.tensor_tensor(crow[:], t1[:], t2[:], mybir.AluOpType.add)
                nc.scalar.activation(tc2[:], crow[:], Tanh)
                nc.scalar.activation(so[:], ap_o, Sig)
                nc.vector.tensor_tensor(h2b[:], so[:], tc2[:], mybir.AluOpType.mult)

                nc.sync.dma_start(agin[par][:], h2b[:])
                nc.gpsimd.collective_compute(
                    "AllGather", mybir.AluOpType.bypass,
                    replica_groups=[list(range(NCORES))],
                    ins=[agin[par][:].opt()], outs=[agout[par][:].opt()],
                )
                g8 = sp.tile([8, S], BF16, tag="g8")
                nc.sync.dma_start(g8[:], agout[par][:])
                # two PSUM banks so copy-A overlaps transpose-B (same-bank
                # PE-write/DVE-read would otherwise serialize them)
                tpA = psp.tile([128, 8], BF16, tag="tpA", bufs=1)
                tpB = psp.tile([128, 8], BF16, tag="tpB", bufs=1)
                nc.tensor.transpose(tpA[:], g8[0:8, 0:128], id8[:])
                nc.vector.tensor_copy(hnxt[:, 0:16:2], tpA[:])
                nc.tensor.transpose(tpB[:], g8[0:8, 128:256], id8[:])
                nc.vector.tensor_copy(hnxt[:, 1:16:2], tpB[:])

                lp = psp.tile([1, V], F32, tag="lp", bufs=1)
                for c in range(KC):
                    nc.tensor.matmul(
                        lp[0:1, :],
                        lhsT=hnxt[:, c:c + 1],
                        rhs=wo[:, V * c: V * (c + 1)],
                        start=(c == 0), stop=(c == KC - 1),
                    )
                nc.vector.tensor_copy(lstore[0:1, t, :], lp[0:1, :])
                mx = sp.tile([1, 1], F32, tag="mx")
                ohrow = sp.tile([1, V], BF16, tag="ohrow")
                ohcol = psp.tile([V, 1], BF16, tag="ohcol", bufs=1)
                nc.vector.reduce_max(mx[:], lp[0:1, :], mybir.AxisListType.X)
                nc.vector.tensor_scalar(ohrow[:], lp[0:1, :], mx[:], None,
                                        mybir.AluOpType.is_equal)
                nc.tensor.transpose(ohcol[:], ohrow[:], ident[:])
                nc.vector.tensor_copy(hnxt[0:V, 16:17], ohcol[:])

                if t == T - 1:
                    h2f = sp.tile([1, S], F32, tag="h2f")
                    nc.vector.tensor_tensor(h2f[:], so[:], tc2[:],
                                            mybir.AluOpType.mult)
                    nc.sync.dma_start(fin_in[0:1, 0:S], h2f[:])
                    nc.sync.dma_start(fin_in[0:1, S:2 * S], crow[:])

            nc.gpsimd.collective_compute(
                "AllGather", mybir.AluOpType.bypass,
                replica_groups=[list(range(NCORES))],
                ins=[fin_in[:].opt()], outs=[fin_out[:].opt()],
            )
            nc.sync.dma_start(out_h[:].rearrange("a (r s) -> (a r) s", s=S),
                              fin_out[:, 0:S])
            nc.sync.dma_start(out_c[:].rearrange("a (r s) -> (a r) s", s=S),
                              fin_out[:, S:2 * S])

            mxs = postp.tile([1, T, 1], F32, tag="mxs")
            sub = postp.tile([1, T, V], F32, tag="sub")
            ex = postp.tile([1, T, V], F32, tag="ex")
            sm = postp.tile([1, T, 1], F32, tag="sm")
            lsm = postp.tile([1, T, 1], F32, tag="lsm")
            nc.vector.reduce_max(mxs[0:1, :, 0], lstore[:], mybir.AxisListType.X)
            nc.vector.tensor_tensor(sub[:], lstore[:],
                                    mxs[:].broadcast_to([1, T, V]),
                                    mybir.AluOpType.subtract)
            nc.scalar.activation(ex[:], sub[:], mybir.ActivationFunctionType.Exp)
            nc.vector.reduce_sum(sm[0:1, :, 0], ex[:], mybir.AxisListType.X)
            nc.scalar.activation(lsm[:], sm[:], mybir.ActivationFunctionType.Ln)
            nc.vector.tensor_tensor(sub[:], sub[:],
                                    lsm[:].broadcast_to([1, T, V]),
                                    mybir.AluOpType.subtract)
            nc.sync.dma_start(out_lp[:].rearrange("t v -> (t v)").unsqueeze(0),
                              sub[0:1, :, :])

    nc.compile()
    return nc


def _get_nc():
    if "nc" not in _cache:
        _cache["nc"] = _build_nc()
    return _cache["nc"]


def run_on_device(inputs, trace=False, tmpdir=None):
    from concourse import bass_utils
    nc = _get_nc()
    in_maps = [_pack_inputs(inputs, core) for core in range(NCORES)]
    res = bass_utils.run_bass_kernel_spmd(
        nc, in_maps, core_ids=list(range(NCORES)), trace=trace, tmpdir=tmpdir)
    r0 = res.results[0]
    out = (np.asarray(r0["out_lp"], np.float32),
           np.asarray(r0["out_h"], np.float32),
           np.asarray(r0["out_c"], np.float32))
    return out, res


def kernel(**inputs):
    out, _ = run_on_device(inputs, trace=False)
    return out
